# revision 3
# baseline (speedup 1.0000x reference)
"""ChannelCrossAttention TRN2 Bass kernel.

Reference computation (per batch b):
    q = Wq @ f1 + bq          [C8, N] -> used as q[n, o]
    k = Wk @ f2 + bk          [C8, N]
    v = Wv @ f2 + bv          [C, N]
    energy[n, m] = sum_o q[m, o] k[o, n]     (note: we compute energy TRANSPOSED:
                                              eT[n_key, m_query])
    attn = softmax over keys
    out[c, m] = sum_n v[c, n] attn[m, n]
    result = gamma * out + f1

Sharding: 8 cores; core i handles batch b = i // 2, query half h = i % 2
(2048 query positions each). Each core gets the full feat2[b] (keys/values)
and its query slice of feat1[b].

On-chip layout (per core):
  - energyT computed in [n(partition), m(free)] layout via
    matmul(lhsT=K[o, n_chunk], rhs=Q[o, m_tile]).
  - exp on ScalarE (no max subtraction: |energy| <= ~45 << 88, fp32-safe).
  - softmax denominator S[m] via ones-vector matmul (partition reduction on PE).
  - out[c, m] accumulated as matmul(lhsT=VT[n_chunk, c_chunk], rhs=expT),
    normalized by gamma/S at the end, then + feat1 residual.

All heavy matmuls use float32r (tf32-like, 1 col/cycle vs 4 for fp32).
float32r operands must be produced by a compute engine, so DMA'd inputs get
one DVE rounding pass.
"""

import numpy as np

B, C, H, W = 4, 256, 64, 64
N = H * W            # 4096 keys
C8 = C // 8          # 32
P = 128              # partitions
M = N // 2           # 2048 queries per core
MT = 512             # query tile (PSUM bank = 512 fp32)
NMT = M // MT        # 4
NJ = N // P          # 32 key chunks
CCH = C // P         # 2 channel chunks
NCORES = 8

_cache = {}


def _build_nc():
    import concourse.tile as tile
    from concourse import bacc, mybir

    f32 = mybir.dt.float32
    f32r = mybir.dt.float32r
    Exp = mybir.ActivationFunctionType.Exp

    nc = bacc.Bacc("TRN2", target_bir_lowering=False, debug=False)

    d_f2 = nc.dram_tensor("f2", [C, N], f32, kind="ExternalInput").ap()
    d_f1 = nc.dram_tensor("f1s", [C, M], f32, kind="ExternalInput").ap()
    d_wq = nc.dram_tensor("wqT", [C, C8], f32, kind="ExternalInput").ap()
    d_wk = nc.dram_tensor("wkT", [C, C8], f32, kind="ExternalInput").ap()
    d_wv = nc.dram_tensor("wvT", [C, C], f32, kind="ExternalInput").ap()
    d_bq = nc.dram_tensor("bq", [C8, 1], f32, kind="ExternalInput").ap()
    d_bk = nc.dram_tensor("bk", [C8, 1], f32, kind="ExternalInput").ap()
    d_bvb = nc.dram_tensor("bvb", [P, C], f32, kind="ExternalInput").ap()
    d_grow = nc.dram_tensor("grow", [1, P], f32, kind="ExternalInput").ap()
    d_out = nc.dram_tensor("out", [C, M], f32, kind="ExternalOutput").ap()

    with tile.TileContext(nc) as tc:
        with tc.tile_pool(name="consts", bufs=1) as consts:
            # ---- persistent SBUF tensors ----
            f2r = consts.tile([P, CCH, N], f32r)       # rounded feat2
            f1raw = consts.tile([P, CCH, M], f32)      # full-precision residual
            f1r = consts.tile([P, CCH, M], f32r)       # rounded for Q proj
            wq_sb = consts.tile([P, CCH, C8], f32r)
            wk_sb = consts.tile([P, CCH, C8], f32r)
            wv_sb = consts.tile([P, CCH, C], f32r)
            bq_sb = consts.tile([C8, 1], f32)
            bk_sb = consts.tile([C8, 1], f32)
            bvb_sb = consts.tile([P, C], f32)
            grow_sb = consts.tile([1, P], f32)
            ones_f32 = consts.tile([P, 1], f32)
            ones_sb = consts.tile([P, 1], f32r)
            Q_sb = consts.tile([C8, M], f32r)
            K_sb = consts.tile([C8, N], f32r)
            VT_sb = consts.tile([P, NJ, C], f32r)

            nc.vector.memset(ones_f32, 1.0)
            nc.vector.tensor_copy(ones_sb, ones_f32)
            nc.sync.dma_start(out=f1raw[:, 0, :], in_=d_f1[0:P, :])
            nc.sync.dma_start(out=f1raw[:, 1, :], in_=d_f1[P:C, :])
            nc.sync.dma_start(out=bq_sb, in_=d_bq)
            nc.sync.dma_start(out=bk_sb, in_=d_bk)
            nc.sync.dma_start(out=bvb_sb, in_=d_bvb)
            nc.sync.dma_start(out=grow_sb, in_=d_grow)

            # ---- load + round inputs, projections ----
            with tc.tile_pool(name="stage", bufs=2) as stage, \
                 tc.tile_pool(name="wstage", bufs=2) as wstage, \
                 tc.tile_pool(name="proj_ps", space="PSUM", bufs=2) as pps:

                for ci in range(CCH):
                    st = stage.tile([P, N], f32, tag="st", bufs=2)
                    nc.sync.dma_start(out=st, in_=d_f2[ci * P:(ci + 1) * P, :])
                    nc.vector.tensor_copy(f2r[:, ci, :], st)
                for ci in range(CCH):
                    nc.vector.tensor_copy(f1r[:, ci, :], f1raw[:, ci, :])

                for dsrc, dst, width in ((d_wq, wq_sb, C8), (d_wk, wk_sb, C8),
                                         (d_wv, wv_sb, C)):
                    for ci in range(CCH):
                        ws = wstage.tile([P, C], f32, tag="ws", bufs=2)
                        nc.sync.dma_start(out=ws[:, 0:width],
                                          in_=dsrc[ci * P:(ci + 1) * P, :])
                        nc.vector.tensor_copy(dst[:, ci, :], ws[:, 0:width])

                # Q[o, m] = sum_c WqT[c, o]^T f1[c, m]  (+bq)
                for mt in range(NMT):
                    q_ps = pps.tile([C8, MT], f32, tag="qk", bufs=2)
                    for ci in range(CCH):
                        nc.tensor.matmul(q_ps, lhsT=wq_sb[:, ci, :],
                                         rhs=f1r[:, ci, mt * MT:(mt + 1) * MT],
                                         start=(ci == 0), stop=(ci == CCH - 1))
                    nc.vector.tensor_scalar_add(Q_sb[:, mt * MT:(mt + 1) * MT],
                                                q_ps, bq_sb)
                # K[o, n] likewise from f2
                for nt in range(N // MT):
                    k_ps = pps.tile([C8, MT], f32, tag="qk", bufs=2)
                    for ci in range(CCH):
                        nc.tensor.matmul(k_ps, lhsT=wk_sb[:, ci, :],
                                         rhs=f2r[:, ci, nt * MT:(nt + 1) * MT],
                                         start=(ci == 0), stop=(ci == CCH - 1))
                    nc.vector.tensor_scalar_add(K_sb[:, nt * MT:(nt + 1) * MT],
                                                k_ps, bk_sb)
                # VT[n, c] = sum_ci f2[ci, n]^T WvT[ci, c]  (+bv broadcast)
                for nj in range(NJ):
                    v_ps = pps.tile([P, C], f32, tag="v", bufs=2)
                    for ci in range(CCH):
                        nc.tensor.matmul(v_ps,
                                         lhsT=f2r[:, ci, nj * P:(nj + 1) * P],
                                         rhs=wv_sb[:, ci, :],
                                         start=(ci == 0), stop=(ci == CCH - 1))
                    nc.vector.tensor_add(VT_sb[:, nj, :], v_ps, bvb_sb)

            # ---- attention main loop ----
            with tc.tile_pool(name="main_ps", space="PSUM", bufs=1) as mps, \
                 tc.tile_pool(name="expool", bufs=6) as expool, \
                 tc.tile_pool(name="opool", bufs=2) as opool:
                for mt in range(NMT):
                    ms = slice(mt * MT, (mt + 1) * MT)
                    out_ps = []
                    for cch in range(CCH):
                        o_ps = mps.tile([P, MT], f32, tag=f"out{cch}", bufs=2,
                                        name=f"o_ps{cch}")
                        out_ps.append(o_ps)
                    s_ps = mps.tile([1, MT], f32, tag="s", bufs=1)

                    for nj in range(NJ):
                        e_ps = mps.tile([P, MT], f32, tag="e", bufs=2)
                        nc.tensor.matmul(e_ps,
                                         lhsT=K_sb[:, nj * P:(nj + 1) * P],
                                         rhs=Q_sb[:, ms],
                                         start=True, stop=True)
                        ex = expool.tile([P, MT], f32r, tag="ex", bufs=6)
                        nc.scalar.activation(ex, e_ps, Exp)
                        nc.tensor.matmul(s_ps, lhsT=ones_sb, rhs=ex,
                                         start=(nj == 0), stop=(nj == NJ - 1))
                        for cch in range(CCH):
                            nc.tensor.matmul(out_ps[cch],
                                             lhsT=VT_sb[:, nj,
                                                        cch * P:(cch + 1) * P],
                                             rhs=ex,
                                             start=(nj == 0),
                                             stop=(nj == NJ - 1))

                    # normalize: out * gamma / S, plus residual
                    s_sb = opool.tile([1, MT], f32, tag="s_sb", bufs=2)
                    nc.vector.tensor_copy(s_sb, s_ps)
                    srow = opool.tile([1, MT], f32, tag="srow", bufs=2)
                    scr = opool.tile([1, MT], f32, tag="scr", bufs=2)
                    nc.vector.reciprocal_approx_accurate(out=srow, in_=s_sb,
                                                         scratch=scr)
                    rg_ps = mps.tile([P, MT], f32, tag="rg", bufs=1)
                    nc.tensor.matmul(rg_ps, lhsT=grow_sb, rhs=srow,
                                     start=True, stop=True)
                    rg_sb = opool.tile([P, MT], f32, tag="rg_sb", bufs=2)
                    nc.vector.tensor_copy(rg_sb, rg_ps)
                    for cch in range(CCH):
                        t_sb = opool.tile([P, MT], f32, tag=f"t{cch}", bufs=2)
                        nc.vector.tensor_mul(t_sb, out_ps[cch], rg_sb)
                        o_sb = opool.tile([P, MT], f32, tag=f"o{cch}", bufs=2)
                        nc.vector.tensor_add(o_sb, t_sb, f1raw[:, cch, ms])
                        nc.sync.dma_start(
                            out=d_out[cch * P:(cch + 1) * P, ms], in_=o_sb)

    nc.compile()
    return nc


def _get_nc():
    if "nc" not in _cache:
        _cache["nc"] = _build_nc()
    return _cache["nc"]


def kernel(feat1, feat2, Wq, bq, Wk, bk, Wv, bv, gamma, _trace=False):
    from concourse.bass_utils import run_bass_kernel_spmd

    feat1 = np.ascontiguousarray(np.asarray(feat1, dtype=np.float32))
    feat2 = np.ascontiguousarray(np.asarray(feat2, dtype=np.float32))
    f1v = feat1.reshape(B, C, N)
    f2v = feat2.reshape(B, C, N)
    wqT = np.ascontiguousarray(np.asarray(Wq, np.float32).T)      # [C, C8]
    wkT = np.ascontiguousarray(np.asarray(Wk, np.float32).T)      # [C, C8]
    wvT = np.ascontiguousarray(np.asarray(Wv, np.float32).T)      # [C, C]
    bqc = np.ascontiguousarray(np.asarray(bq, np.float32).reshape(C8, 1))
    bkc = np.ascontiguousarray(np.asarray(bk, np.float32).reshape(C8, 1))
    bvb = np.ascontiguousarray(
        np.broadcast_to(np.asarray(bv, np.float32)[None, :], (P, C)))
    g = float(np.asarray(gamma, np.float32).reshape(-1)[0])
    grow = np.full((1, P), g, dtype=np.float32)

    nc = _get_nc()
    in_maps = []
    for core in range(NCORES):
        b, half = core // 2, core % 2
        m0 = half * M
        in_maps.append({
            "f2": np.ascontiguousarray(f2v[b]),
            "f1s": np.ascontiguousarray(f1v[b][:, m0:m0 + M]),
            "wqT": wqT, "wkT": wkT, "wvT": wvT,
            "bq": bqc, "bk": bkc, "bvb": bvb, "grow": grow,
        })

    res = run_bass_kernel_spmd(nc, in_maps, core_ids=list(range(NCORES)),
                               trace=_trace)
    _cache["last_result"] = res

    out = np.empty((B, C, N), dtype=np.float32)
    for core in range(NCORES):
        b, half = core // 2, core % 2
        m0 = half * M
        out[b][:, m0:m0 + M] = res.results[core]["out"]
    return out.reshape(B, C, H, W)


# revision 4
# speedup vs baseline: 1.0373x; 1.0373x over previous
"""ChannelCrossAttention TRN2 Bass kernel.

Reference computation (per batch b):
    q = Wq @ f1 + bq          [C8, N]
    k = Wk @ f2 + bk          [C8, N]
    v = Wv @ f2 + bv          [C, N]
    energy[m, n] = q[:, m] . k[:, n]   (computed TRANSPOSED: eT[n_key, m_query])
    attn = softmax over keys n
    out[c, m] = sum_n v[c, n] attn[m, n]
    result = gamma * out + f1

Sharding: 8 cores; core i handles batch b = i // 2, query half h = i % 2
(2048 query positions each). Full feat2[b] (keys/values) per core.

Kernel structure (per core):
  - Q/K are built 4x-replicated across partition blocks (Q4/K4 [128, m]),
    enabling 4x row-packed energy matmuls (K=32 contraction per row group).
  - energyT in [n(partition), m(free)] layout; exp on ScalarE over
    [128, 1024] PSUM pairs; no max subtraction (|energy| <= ~45 << 88).
  - softmax denominator S[m] via ones-vector matmuls (ping-ponged
    stationaries: identical consecutive stationaries serialize the PE).
  - out[c, m] += VT[n, c].T @ expT accumulated over n chunks, then
    normalized by gamma/S and added to the feat1 residual.

All heavy matmuls run in float32r (tf32-like, 1 col/cycle vs 4 for fp32);
float32r operands must be produced by a compute engine, so DMA'd inputs get
one DVE rounding pass.
"""

import numpy as np

B, C, H, W = 4, 256, 64, 64
N = H * W            # 4096 keys
C8 = C // 8          # 32
P = 128              # partitions
M = N // 2           # 2048 queries per core
MT = 512             # query tile (PSUM bank = 512 fp32)
NMT = M // MT        # 4
NJ = N // P          # 32 key chunks
CCH = C // P         # 2 channel chunks
NCORES = 8

_cache = {}


def _build_nc():
    import concourse.tile as tile
    from concourse import bacc, mybir

    f32 = mybir.dt.float32
    f32r = mybir.dt.float32r
    Exp = mybir.ActivationFunctionType.Exp

    nc = bacc.Bacc("TRN2", target_bir_lowering=False, debug=False)

    d_f2 = nc.dram_tensor("f2", [C, N], f32, kind="ExternalInput").ap()
    d_f1 = nc.dram_tensor("f1s", [C, M], f32, kind="ExternalInput").ap()
    d_wq4 = nc.dram_tensor("wq4", [C, P], f32, kind="ExternalInput").ap()
    d_wk4 = nc.dram_tensor("wk4", [C, P], f32, kind="ExternalInput").ap()
    d_wv = nc.dram_tensor("wvT", [C, C], f32, kind="ExternalInput").ap()
    d_bq4 = nc.dram_tensor("bq4", [P, 1], f32, kind="ExternalInput").ap()
    d_bk4 = nc.dram_tensor("bk4", [P, 1], f32, kind="ExternalInput").ap()
    d_bvb = nc.dram_tensor("bvb", [P, C], f32, kind="ExternalInput").ap()
    d_grow = nc.dram_tensor("grow", [1, P], f32, kind="ExternalInput").ap()
    d_out = nc.dram_tensor("out", [C, M], f32, kind="ExternalOutput").ap()

    with tile.TileContext(nc) as tc:
        with tc.tile_pool(name="consts", bufs=1) as consts:
            # ---- persistent SBUF tensors ----
            f2r = consts.tile([P, CCH, N], f32r)       # rounded feat2
            f1raw = consts.tile([P, CCH, M], f32)      # full-precision residual
            f1r = consts.tile([P, CCH, M], f32r)       # rounded for Q proj
            wq4_sb = consts.tile([P, CCH, P], f32r)
            wk4_sb = consts.tile([P, CCH, P], f32r)
            wv_sb = consts.tile([P, CCH, C], f32r)
            bq4_sb = consts.tile([P, 1], f32)
            bk4_sb = consts.tile([P, 1], f32)
            bvb_sb = consts.tile([P, C], f32)
            grow_sb = consts.tile([1, P], f32)
            grow2_sb = consts.tile([1, P], f32)
            ones_f32 = consts.tile([P, 1], f32)
            ones_a = consts.tile([P, 1], f32r)
            ones_b = consts.tile([P, 1], f32r)
            Q4_sb = consts.tile([P, M], f32r)
            K4_sb = consts.tile([P, N], f32r)
            VT_sb = consts.tile([P, NJ, C], f32r)

            nc.vector.memset(ones_f32, 1.0)
            nc.vector.tensor_copy(ones_a, ones_f32)
            nc.vector.tensor_copy(ones_b, ones_f32)
            nc.sync.dma_start(out=f1raw[:, 0, :], in_=d_f1[0:P, :])
            nc.sync.dma_start(out=f1raw[:, 1, :], in_=d_f1[P:C, :])
            nc.sync.dma_start(out=bq4_sb, in_=d_bq4)
            nc.sync.dma_start(out=bk4_sb, in_=d_bk4)
            nc.sync.dma_start(out=bvb_sb, in_=d_bvb)
            nc.sync.dma_start(out=grow_sb, in_=d_grow)
            nc.sync.dma_start(out=grow2_sb, in_=d_grow)

            # ---- load + round inputs, projections ----
            with tc.tile_pool(name="stage", bufs=2) as stage, \
                 tc.tile_pool(name="proj_ps", space="PSUM", bufs=2) as pps:

                for ci in range(CCH):
                    st = stage.tile([P, N], f32, tag="st", bufs=2)
                    nc.sync.dma_start(out=st, in_=d_f2[ci * P:(ci + 1) * P, :])
                    nc.vector.tensor_copy(f2r[:, ci, :], st)
                for ci in range(CCH):
                    nc.vector.tensor_copy(f1r[:, ci, :], f1raw[:, ci, :])

                for dsrc, dst, width in ((d_wq4, wq4_sb, P), (d_wk4, wk4_sb, P),
                                         (d_wv, wv_sb, C)):
                    for ci in range(CCH):
                        ws = stage.tile([P, C], f32, tag="ws", bufs=2,
                                        name="ws")
                        nc.sync.dma_start(out=ws[:, 0:width],
                                          in_=dsrc[ci * P:(ci + 1) * P, :])
                        nc.vector.tensor_copy(dst[:, ci, :], ws[:, 0:width])

                # Q4[o4, m]: 4x-replicated Q
                for mt in range(NMT):
                    q_ps = pps.tile([P, MT], f32, tag="qk", bufs=2)
                    for ci in range(CCH):
                        nc.tensor.matmul(q_ps, lhsT=wq4_sb[:, ci, :],
                                         rhs=f1r[:, ci, mt * MT:(mt + 1) * MT],
                                         start=(ci == 0), stop=(ci == CCH - 1))
                    nc.vector.tensor_scalar_add(Q4_sb[:, mt * MT:(mt + 1) * MT],
                                                q_ps, bq4_sb)
                # K4[o4, n]: 4x-replicated K
                for nt in range(N // MT):
                    k_ps = pps.tile([P, MT], f32, tag="qk", bufs=2)
                    for ci in range(CCH):
                        nc.tensor.matmul(k_ps, lhsT=wk4_sb[:, ci, :],
                                         rhs=f2r[:, ci, nt * MT:(nt + 1) * MT],
                                         start=(ci == 0), stop=(ci == CCH - 1))
                    nc.vector.tensor_scalar_add(K4_sb[:, nt * MT:(nt + 1) * MT],
                                                k_ps, bk4_sb)
                # VT[n, c] = sum_ci f2[ci, n]^T WvT[ci, c]  (+bv broadcast)
                for nj in range(NJ):
                    v_ps = pps.tile([P, C], f32, tag="v", bufs=2)
                    for ci in range(CCH):
                        nc.tensor.matmul(v_ps,
                                         lhsT=f2r[:, ci, nj * P:(nj + 1) * P],
                                         rhs=wv_sb[:, ci, :],
                                         start=(ci == 0), stop=(ci == CCH - 1))
                    nc.vector.tensor_add(VT_sb[:, nj, :], v_ps, bvb_sb)

            # ---- attention main loop ----
            with tc.tile_pool(name="main_ps", space="PSUM", bufs=1) as mps, \
                 tc.tile_pool(name="expool", bufs=3) as expool, \
                 tc.tile_pool(name="opool", bufs=2) as opool:
                for mt in range(NMT):
                    ms = slice(mt * MT, (mt + 1) * MT)
                    out_ps = []
                    for cch in range(CCH):
                        o_ps = mps.tile([P, MT], f32, tag=f"out{cch}", bufs=1,
                                        name=f"o_ps{cch}")
                        out_ps.append(o_ps)
                    s_ps = mps.tile([1, MT], f32, tag="s", bufs=2)

                    for g in range(NJ // 4):
                        # 4x row-packed energy into two [128, 1024] psum pairs
                        e_pair = [
                            mps.tile([P, 2, MT], f32, tag=f"e{h}", bufs=1,
                                     name=f"e_pair{h}")
                            for h in range(2)
                        ]
                        for i in range(4):
                            nj = 4 * g + i
                            nc.tensor.matmul(
                                e_pair[i // 2][:, i % 2, :],
                                lhsT=K4_sb[32 * i:32 * (i + 1),
                                           nj * P:(nj + 1) * P],
                                rhs=Q4_sb[32 * i:32 * (i + 1), ms],
                                start=True, stop=True,
                                tile_position=(32 * i, 0),
                            )
                        ex_pair = []
                        for h in range(2):
                            ex = expool.tile([P, 2, MT], f32r, tag="ex",
                                             bufs=3, name="ex")
                            nc.scalar.activation(ex, e_pair[h], Exp)
                            ex_pair.append(ex)
                        for i in range(4):
                            nj = 4 * g + i
                            ex = ex_pair[i // 2][:, i % 2, :]
                            for cch in range(CCH):
                                nc.tensor.matmul(
                                    out_ps[cch],
                                    lhsT=VT_sb[:, nj, cch * P:(cch + 1) * P],
                                    rhs=ex,
                                    start=(nj == 0), stop=(nj == NJ - 1),
                                )
                            nc.tensor.matmul(
                                s_ps, lhsT=(ones_a if nj % 2 == 0 else ones_b),
                                rhs=ex,
                                start=(nj == 0), stop=(nj == NJ - 1),
                            )

                    # normalize: out * gamma / S, plus residual
                    s_sb = opool.tile([1, MT], f32, tag="s_sb", bufs=2)
                    nc.vector.tensor_copy(s_sb, s_ps)
                    srow = opool.tile([1, MT], f32, tag="srow", bufs=2)
                    scr = opool.tile([1, MT], f32, tag="scr", bufs=2)
                    nc.vector.reciprocal_approx_accurate(out=srow, in_=s_sb,
                                                         scratch=scr)
                    rg_ps = mps.tile([P, MT], f32, tag="e0", bufs=1,
                                     name="rg_ps")
                    nc.tensor.matmul(rg_ps,
                                     lhsT=(grow_sb if mt % 2 == 0 else grow2_sb),
                                     rhs=srow, start=True, stop=True)
                    rg_sb = opool.tile([P, MT], f32, tag="rg_sb", bufs=2)
                    nc.vector.tensor_copy(rg_sb, rg_ps)
                    for cch in range(CCH):
                        t_sb = opool.tile([P, MT], f32, tag=f"t{cch}", bufs=2)
                        nc.vector.tensor_mul(t_sb, out_ps[cch], rg_sb)
                        o_sb = opool.tile([P, MT], f32, tag=f"o{cch}", bufs=2)
                        nc.vector.tensor_add(o_sb, t_sb, f1raw[:, cch, ms])
                        nc.sync.dma_start(
                            out=d_out[cch * P:(cch + 1) * P, ms], in_=o_sb)

    nc.compile()
    return nc


def _get_nc():
    if "nc" not in _cache:
        _cache["nc"] = _build_nc()
    return _cache["nc"]


def kernel(feat1, feat2, Wq, bq, Wk, bk, Wv, bv, gamma, _trace=False):
    from concourse.bass_utils import run_bass_kernel_spmd

    feat1 = np.ascontiguousarray(np.asarray(feat1, dtype=np.float32))
    feat2 = np.ascontiguousarray(np.asarray(feat2, dtype=np.float32))
    f1v = feat1.reshape(B, C, N)
    f2v = feat2.reshape(B, C, N)
    wqT = np.asarray(Wq, np.float32).T                            # [C, C8]
    wkT = np.asarray(Wk, np.float32).T
    wq4 = np.ascontiguousarray(np.concatenate([wqT] * 4, axis=1))  # [C, 128]
    wk4 = np.ascontiguousarray(np.concatenate([wkT] * 4, axis=1))
    bq4 = np.ascontiguousarray(np.tile(np.asarray(bq, np.float32), 4)[:, None])
    bk4 = np.ascontiguousarray(np.tile(np.asarray(bk, np.float32), 4)[:, None])
    wvT = np.ascontiguousarray(np.asarray(Wv, np.float32).T)      # [C, C]
    bvb = np.ascontiguousarray(
        np.broadcast_to(np.asarray(bv, np.float32)[None, :], (P, C)))
    g = float(np.asarray(gamma, np.float32).reshape(-1)[0])
    grow = np.full((1, P), g, dtype=np.float32)

    nc = _get_nc()
    in_maps = []
    for core in range(NCORES):
        b, half = core // 2, core % 2
        m0 = half * M
        in_maps.append({
            "f2": np.ascontiguousarray(f2v[b]),
            "f1s": np.ascontiguousarray(f1v[b][:, m0:m0 + M]),
            "wq4": wq4, "wk4": wk4, "wvT": wvT,
            "bq4": bq4, "bk4": bk4, "bvb": bvb, "grow": grow,
        })

    res = run_bass_kernel_spmd(nc, in_maps, core_ids=list(range(NCORES)),
                               trace=_trace)
    _cache["last_result"] = res

    out = np.empty((B, C, N), dtype=np.float32)
    for core in range(NCORES):
        b, half = core // 2, core % 2
        m0 = half * M
        out[b][:, m0:m0 + M] = res.results[core]["out"]
    return out.reshape(B, C, H, W)
